# revision 31
# baseline (speedup 1.0000x reference)
# Multi-head attention (B=4, T=2048, D=1024, H=16, dqk=dv=64) on 8 trn2
# NeuronCores. Sharding: core c -> batch c//2, head-group c%2 (8 heads).
# Each core computes its batch's Q^T/K^T/V projections for its heads,
# causal flash attention with transposed scores (S^T[kv,q]; softmax
# normalizer via a ones-column appended to V), and a partial output
# projection. Host sums the two partials per batch and adds biases.
import numpy as np
import ml_dtypes

B, TQ, TKV, DM, H, DQ, DV = 4, 2048, 2048, 1024, 16, 64, 64
NC = 8          # cores
HL = 8          # heads per core
NHP = HL // 2   # 128-partition head-pair tiles (4)
SB = 512        # q super-block width
NQSB = TQ // SB
NKT = TKV // 128
NDM = DM // 128
P = 128

bf16 = ml_dtypes.bfloat16

_programs = {}
_last_in_maps = None


def _make_tc_class(tile_mod):
    from concourse.vector_clock import ScopedClock
    import concourse.mybir as mybir

    class TC(tile_mod.TileContext):
        # This toolchain's walrus codegen encodes at most ONE sync wait
        # per instruction. Tile's wait assignment can attach several, so
        # before lowering, peel extra waits off onto standalone
        # InstEventSemaphore instructions placed immediately before the
        # instruction on the same engine (in-order execution makes this
        # semantically identical).
        def _lower_ordered_insts(self, ordered):
            for bb_name, insts in ordered.items():
                out = []
                for inst in insts:
                    si = getattr(inst, "sync_info", None)
                    eng = getattr(inst, "engine", None)
                    if (
                        si is not None
                        and si.on_wait
                        and len(si.on_wait) > 1
                        and eng is not None
                        and eng != mybir.EngineType.Unassigned
                    ):
                        waits = list(si.on_wait)
                        for w in waits[:-1]:
                            ev = mybir.InstEventSemaphore(
                                name=f"I-{self.nc.next_id()}", ins=[], outs=[]
                            )
                            ev.engine = eng
                            ev.sync_info = mybir.SyncInfo(
                                on_wait=[w], on_update=[]
                            )
                            out.append(ev)
                        si.on_wait = waits[-1:]
                    out.append(inst)
                insts[:] = out
            return super()._lower_ordered_insts(ordered)

        # Same 1-wait limit applies to the tail drain; split its waits
        # into standalone wait instructions.
        def _drain_and_barrier(self, tick_clock, wait_clock):
            drain_inst = self.nc.sync.drain()
            wait_clock.add_sem_waits(
                drain_inst.ins, ScopedClock({None: tick_clock.global_clock})
            )
            si = drain_inst.ins.sync_info
            waits = list(si.on_wait) if si and si.on_wait else []
            if len(waits) > 1:
                si.on_wait = waits[:1]
                name2sem = {}
                for s in self.sems.allocated().values():
                    name2sem[getattr(s, "name", None) or str(s)] = s
                for w in waits[1:]:
                    self.nc.sync.wait_ge(name2sem[w.ant_name], w.wait_value)
            self.nc.all_engine_barrier()
            popped = self.nc._tile_sem_poison_stack.pop()
            assert popped is self._sem_poison
            self.nc.clear_and_free_semaphores(list(self.sems.allocated().values()))
            self.nc.all_engine_barrier()

    return TC


def build_program(causal: bool):
    import concourse.bass as bass
    import concourse.mybir as mybir
    import concourse.tile as tile

    dt = mybir.dt
    AF = mybir.ActivationFunctionType
    TC = _make_tc_class(tile)

    nc = bass.Bass("TRN2", target_bir_lowering=False, debug=False, num_devices=NC)

    xqT = nc.dram_tensor("xqT", [DM, TQ], dt.bfloat16, kind="ExternalInput")
    xkvT = nc.dram_tensor("xkvT", [DM, TKV], dt.bfloat16, kind="ExternalInput")
    wq_d = nc.dram_tensor("wq", [DM, HL * DQ], dt.bfloat16, kind="ExternalInput")
    wk_d = nc.dram_tensor("wk", [DM, HL * DQ], dt.bfloat16, kind="ExternalInput")
    wv_d = nc.dram_tensor("wv", [DM, HL * DV], dt.bfloat16, kind="ExternalInput")
    wo_d = nc.dram_tensor("wo", [HL * DV, DM], dt.bfloat16, kind="ExternalInput")
    bq_d = nc.dram_tensor("bqp", [P, NHP], dt.float32, kind="ExternalInput")
    bk_d = nc.dram_tensor("bkp", [P, NHP], dt.float32, kind="ExternalInput")
    pad_d = nc.dram_tensor("pad", [P, NKT], dt.float32, kind="ExternalInput")
    msk_d = nc.dram_tensor("msk", [P, 4 * SB], dt.bfloat16, kind="ExternalInput")
    one_d = nc.dram_tensor("one64", [P, HL * 64], dt.bfloat16, kind="ExternalInput")
    out_d = nc.dram_tensor("out", [TQ, DM], dt.float32, kind="ExternalOutput")

    with TC(nc) as tc:
        with (
            tc.tile_pool(name="res", bufs=1) as res,
            tc.tile_pool(name="xp", bufs=8) as xp,
            tc.tile_pool(name="ptp", bufs=4) as ptp,
            tc.tile_pool(name="atp", bufs=2) as atp,
            tc.tile_pool(name="rcp", bufs=2) as rcp,
            tc.tile_pool(name="ps_proj", bufs=2, space="PSUM") as ps_proj,
            tc.tile_pool(name="ps_s", bufs=3, space="PSUM") as ps_s,
            tc.tile_pool(name="ps_at", bufs=2, space="PSUM") as ps_at,
            tc.tile_pool(name="ps_bc", bufs=1, space="PSUM") as ps_bc,
        ):
            # ---- resident constants / weights ----
            wq_t, wk_t, wv_t = [], [], []
            for k in range(NDM):
                t = res.tile([P, HL * DQ], dt.bfloat16, tag=f"wq{k}", name=f"wq{k}")
                nc.sync.dma_start(t[:], wq_d.ap()[k * P:(k + 1) * P, :])
                wq_t.append(t)
                t = res.tile([P, HL * DQ], dt.bfloat16, tag=f"wk{k}", name=f"wk{k}")
                nc.sync.dma_start(t[:], wk_d.ap()[k * P:(k + 1) * P, :])
                wk_t.append(t)
                t = res.tile([P, HL * DV], dt.bfloat16, tag=f"wv{k}", name=f"wv{k}")
                nc.sync.dma_start(t[:], wv_d.ap()[k * P:(k + 1) * P, :])
                wv_t.append(t)
            wo_t = []
            for hp in range(NHP):
                t = res.tile([P, DM], dt.bfloat16, tag=f"wo{hp}", name=f"wo{hp}")
                nc.sync.dma_start(t[:], wo_d.ap()[hp * P:(hp + 1) * P, :])
                wo_t.append(t)
            bq_t = res.tile([P, NHP], dt.float32, tag="bq", name="bq_t")
            nc.sync.dma_start(bq_t[:], bq_d.ap()[:, :])
            bk_t = res.tile([P, NHP], dt.float32, tag="bk", name="bk_t")
            nc.sync.dma_start(bk_t[:], bk_d.ap()[:, :])
            pad_t = res.tile([P, NKT], dt.float32, tag="pad", name="pad_t")
            nc.sync.dma_start(pad_t[:], pad_d.ap()[:, :])
            msk_t = res.tile([P, 4 * SB], dt.bfloat16, tag="msk", name="msk_t")
            nc.sync.dma_start(msk_t[:], msk_d.ap()[:, :])
            sel_t = res.tile([P, HL * 64], dt.bfloat16, tag="sel", name="sel_t")
            nc.sync.dma_start(sel_t[:], one_d.ap()[:, :])


            # ---- x^T streams (xq slots reused by xkv) ----
            xq_t = []
            for k in range(NDM):
                t = xp.tile([P, TQ], dt.bfloat16, tag="xT", name="xT")
                nc.sync.dma_start(t[:], xqT.ap()[k * P:(k + 1) * P, :])
                xq_t.append(t)

            # ---- Q^T projection: qT[hp] = [128(2 heads x 64), TQ] bf16 ----
            qT = [res.tile([P, TQ], dt.bfloat16, tag=f"qT{hp}", name=f"qT{hp}") for hp in range(NHP)]
            for qsb in range(NQSB):
                for hp in range(NHP):
                    ps = ps_proj.tile([P, SB], dt.float32, tag="pp", name="pp")
                    for k in range(NDM):
                        nc.tensor.matmul(
                            ps[:],
                            wq_t[k][:, hp * P:(hp + 1) * P],
                            xq_t[k][:, qsb * SB:(qsb + 1) * SB],
                            start=(k == 0), stop=(k == NDM - 1),
                        )
                    with nc.allow_low_precision(reason="bf16 Q"):
                        nc.vector.tensor_scalar_add(
                            qT[hp][:, qsb * SB:(qsb + 1) * SB], ps[:],
                            bq_t[:, hp:hp + 1],
                        )

            xkv_t = []
            for k in range(NDM):
                t = xp.tile([P, TKV], dt.bfloat16, tag="xT", name="xT")
                nc.sync.dma_start(t[:], xkvT.ap()[k * P:(k + 1) * P, :])
                xkv_t.append(t)

            # ---- K^T and V projections (kv-chunk major so attention can
            # start early). V layout: per kv tile [128, 8 heads x 65]
            # (64 V cols + 1 ones col per head) for the PV+rowsum matmul. ----
            kT = [res.tile([P, TKV], dt.bfloat16, tag=f"kT{hp}", name=f"kT{hp}") for hp in range(NHP)]
            v_t = [res.tile([P, HL * 65], dt.bfloat16, tag=f"v{vt}", name=f"v{vt}") for vt in range(NKT)]
            for nj in range(NQSB):
                for hp in range(NHP):
                    ps = ps_proj.tile([P, SB], dt.float32, tag="pp", name="pp")
                    for k in range(NDM):
                        nc.tensor.matmul(
                            ps[:],
                            wk_t[k][:, hp * P:(hp + 1) * P],
                            xkv_t[k][:, nj * SB:(nj + 1) * SB],
                            start=(k == 0), stop=(k == NDM - 1),
                        )
                    with nc.allow_low_precision(reason="bf16 K"):
                        nc.vector.tensor_scalar_add(
                            kT[hp][:, nj * SB:(nj + 1) * SB], ps[:],
                            bk_t[:, hp:hp + 1],
                        )
                for vt in range(nj * 4, nj * 4 + 4):
                    ps = ps_proj.tile([P, SB], dt.float32, tag="pp", name="pp")
                    for k in range(NDM):
                        nc.tensor.matmul(
                            ps[:],
                            xkv_t[k][:, vt * P:(vt + 1) * P],
                            wv_t[k][:],
                            start=(k == 0), stop=(k == NDM - 1),
                        )
                    vtile = v_t[vt]
                    nc.vector.tensor_copy(
                        vtile[:].rearrange("p (h d) -> p h d", d=65)[:, :, 0:64],
                        ps[:].rearrange("p (h d) -> p h d", d=64),
                    )
                    nc.gpsimd.memset(
                        vtile[:].rearrange("p (h d) -> p h d", d=65)[:, :, 64:65], 1.0
                    )

            # ---- attention + output projection, per q super-block ----
            def emit_finalize(qsb, at_tiles, an_tiles, rc):
                for h in range(HL):
                    hp, off = h // 2, (h % 2) * 64
                    # broadcast head h's recip row across 64 partitions via
                    # PE: sel[:, h*64:(h+1)*64] is 1.0 on row 32*(h%4)
                    bc = ps_bc.tile([64, SB], dt.float32, tag="bc", name="bc")
                    cg = (h // 4) * SB
                    nc.tensor.matmul(
                        bc[:], sel_t[:, h * 64:(h + 1) * 64],
                        rc[:, cg:cg + SB],
                        start=True, stop=True,
                    )
                    nc.vector.tensor_mul(
                        at_tiles[hp][off:off + 64, :], an_tiles[h][:], bc[:]
                    )

            def emit_outproj(qsb, at_tiles):
                for qt in range(4):
                    for col in range(2):
                        ps = ps_proj.tile([P, SB], dt.float32, tag="pp", name="pp")
                        for hp in range(NHP):
                            nc.tensor.matmul(
                                ps[:],
                                at_tiles[hp][:, qt * P:(qt + 1) * P],
                                wo_t[hp][:, col * SB:(col + 1) * SB],
                                start=(hp == 0), stop=(hp == NHP - 1),
                            )
                        ost = rcp.tile([P, SB], dt.float32, tag="ost", name="ost", bufs=3)
                        nc.vector.tensor_copy(ost[:], ps[:])
                        r0 = qsb * SB + qt * P
                        nc.sync.dma_start(
                            out_d.ap()[r0:r0 + P, col * SB:(col + 1) * SB], ost[:]
                        )

            prev_blk = None
            for qsb in range(NQSB):
                at_tiles = [
                    atp.tile([P, SB], dt.bfloat16, tag=f"attnT{hp}", name=f"attnT{hp}")
                    for hp in range(NHP)
                ]
                kt_max = 4 * qsb + 4 if causal else NKT
                # normalizer rows gathered at partitions {0,32,64,96} x 2
                # column groups (DVE partition offsets must be 32-aligned);
                # filler 1.0 keeps the reciprocal finite on unused rows
                sums = rcp.tile([P, 2 * SB], dt.float32, tag="sums", name="sums")
                nc.gpsimd.memset(sums[:], 1.0)
                an_tiles = []
                for h in range(HL):
                    hp, off = h // 2, (h % 2) * 64
                    aps = ps_at.tile([65, SB], dt.float32, tag="at", name="at")
                    for kt in range(kt_max):
                        # diagonal blocks (j>=1): only q columns >= 128*j can
                        # be unmasked -> trim the left columns entirely
                        j = kt - 4 * qsb if causal else -1
                        c0 = 128 * j if j > 0 else 0
                        w = SB - c0
                        sps = ps_s.tile([P, SB], dt.float32, tag="s", name="s")
                        nc.tensor.matmul(
                            sps[:, c0:SB],
                            kT[hp][off:off + 64, kt * P:(kt + 1) * P],
                            qT[hp][off:off + 64,
                                   qsb * SB + c0:(qsb + 1) * SB],
                            start=True, stop=True,
                        )
                        pt = ptp.tile([P, SB], dt.bfloat16, tag="pT", name="pT")
                        nc.scalar.activation(
                            pt[:, c0:SB], sps[:, c0:SB], AF.Exp,
                            scale=0.125, bias=pad_t[:, kt:kt + 1],
                        )
                        if causal and j >= 0:
                            nc.vector.tensor_mul(
                                pt[:, c0:SB], pt[:, c0:SB],
                                msk_t[:, j * SB + c0:(j + 1) * SB],
                            )
                        nc.tensor.matmul(
                            aps[:, c0:SB],
                            v_t[kt][:, h * 65:h * 65 + 65],
                            pt[:, c0:SB],
                            start=(kt == 0), stop=(kt == kt_max - 1),
                        )
                    # stage numerator + normalizer row to SBUF, freeing psum
                    an = rcp.tile([64, SB], dt.bfloat16, tag=f"an{h}",
                                  name=f"an{h}")
                    nc.vector.tensor_copy(an[:], aps[0:64, :])
                    r, cg = 32 * (h % 4), (h // 4) * SB
                    nc.vector.tensor_copy(
                        sums[r:r + 1, cg:cg + SB], aps[64:65, :]
                    )
                    an_tiles.append(an)
                # one batched reciprocal for all 8 heads of this q-block
                rc = rcp.tile([P, 2 * SB], dt.bfloat16, tag="rc", name="rc")
                with nc.allow_low_precision(reason="bf16 softmax recip"):
                    nc.vector.reciprocal(rc[:], sums[:])
                # divisions + output projection for the PREVIOUS q block are
                # emitted here, after this block's attention: its reciprocal
                # has long finished, so the PE stream never stalls on it
                if prev_blk is not None:
                    emit_finalize(*prev_blk)
                    emit_outproj(prev_blk[0], prev_blk[1])
                prev_blk = (qsb, at_tiles, an_tiles, rc)
            emit_finalize(*prev_blk)
            emit_outproj(prev_blk[0], prev_blk[1])
    return nc


def _get_program(causal: bool):
    key = bool(causal)
    if key not in _programs:
        _programs[key] = build_program(key)
    return _programs[key]


def kernel(**inputs):
    from concourse.bass_utils import run_bass_kernel_spmd

    xq = np.asarray(inputs["query_sequence"], dtype=np.float32)
    xkv = np.asarray(inputs["key_value_sequence"], dtype=np.float32)
    pmask = np.asarray(inputs["key_value_padding_mask"])
    Wq = np.asarray(inputs["Wq"], dtype=np.float32)
    bq = np.asarray(inputs["bq"], dtype=np.float32)
    Wkv = np.asarray(inputs["Wkv"], dtype=np.float32)
    bkv = np.asarray(inputs["bkv"], dtype=np.float32)
    Wo = np.asarray(inputs["Wo"], dtype=np.float32)
    bo = np.asarray(inputs["bo"], dtype=np.float32)
    causal = bool(np.asarray(inputs["apply_causal_mask"]))

    nc = _get_program(causal)

    Wk_full = Wkv[:, : H * DQ]
    Wv_full = Wkv[:, H * DQ:]
    bk_full = bkv[: H * DQ]
    bv_full = bkv[H * DQ:]

    # causal diagonal masks: mask_j[kv, q] = 1 if q >= kv + 128*j (within a
    # [128 kv, 512 q] block at kv-tile offset j of the q super-block)
    kvi = np.arange(P)[:, None]
    qi = np.arange(SB)[None, :]
    msk = np.concatenate(
        [(qi >= kvi + P * j).astype(np.float32) for j in range(4)], axis=1
    ).astype(bf16)
    # head-row selector: sel[:, h*64:(h+1)*64] = 1.0 on row 32*(h%4) else 0
    sel = np.zeros((P, HL * 64), np.float32)
    for h in range(HL):
        sel[32 * (h % 4), h * 64:(h + 1) * 64] = 1.0
    sel = sel.astype(bf16)

    in_maps = []
    for c in range(NC):
        b, g = divmod(c, 2)
        hs = slice(g * HL * DQ, (g + 1) * HL * DQ)
        pb = np.where(pmask[b], np.float32(-1e30), np.float32(0.0))
        in_maps.append({
            "xqT": np.ascontiguousarray(xq[b].T).astype(bf16),
            "xkvT": np.ascontiguousarray(xkv[b].T).astype(bf16),
            "wq": np.ascontiguousarray(Wq[:, hs]).astype(bf16),
            "wk": np.ascontiguousarray(Wk_full[:, hs]).astype(bf16),
            "wv": np.ascontiguousarray(Wv_full[:, hs]).astype(bf16),
            "wo": np.ascontiguousarray(Wo[hs, :]).astype(bf16),
            "bqp": np.ascontiguousarray(bq[hs].reshape(NHP, P).T),
            "bkp": np.ascontiguousarray(bk_full[hs].reshape(NHP, P).T),
            "pad": np.ascontiguousarray(pb.reshape(NKT, P).T),
            "msk": msk,
            "one64": sel,
        })

    global _last_in_maps
    _last_in_maps = in_maps
    res = run_bass_kernel_spmd(nc, in_maps, core_ids=list(range(NC)))

    host_bias = bo + bv_full @ Wo  # softmax rows sum to 1 -> V-bias is additive
    out = np.empty((B, TQ, DM), np.float32)
    for b in range(B):
        out[b] = res.results[2 * b]["out"] + res.results[2 * b + 1]["out"] + host_bias
    return out


# revision 33
# speedup vs baseline: 1.0571x; 1.0571x over previous
# Multi-head attention (B=4, T=2048, D=1024, H=16, dqk=dv=64) on 8 trn2
# NeuronCores. Sharding: core c -> batch c//2, head-group c%2 (8 heads).
# Each core computes its batch's Q^T/K^T/V projections for its heads,
# causal flash attention with transposed scores (S^T[kv,q]; softmax
# normalizer via a ones-column appended to V), and a partial output
# projection. Host sums the two partials per batch and adds biases.
import numpy as np
import ml_dtypes

B, TQ, TKV, DM, H, DQ, DV = 4, 2048, 2048, 1024, 16, 64, 64
NC = 8          # cores
HL = 8          # heads per core
NHP = HL // 2   # 128-partition head-pair tiles (4)
SB = 512        # q super-block width
NQSB = TQ // SB
NKT = TKV // 128
NDM = DM // 128
P = 128

bf16 = ml_dtypes.bfloat16

_programs = {}
_last_in_maps = None


def _make_tc_class(tile_mod):
    from concourse.vector_clock import ScopedClock
    import concourse.mybir as mybir

    class TC(tile_mod.TileContext):
        # This toolchain's walrus codegen encodes at most ONE sync wait
        # per instruction. Tile's wait assignment can attach several, so
        # before lowering, peel extra waits off onto standalone
        # InstEventSemaphore instructions placed immediately before the
        # instruction on the same engine (in-order execution makes this
        # semantically identical).
        def _lower_ordered_insts(self, ordered):
            for bb_name, insts in ordered.items():
                out = []
                for inst in insts:
                    si = getattr(inst, "sync_info", None)
                    eng = getattr(inst, "engine", None)
                    if (
                        si is not None
                        and si.on_wait
                        and len(si.on_wait) > 1
                        and eng is not None
                        and eng != mybir.EngineType.Unassigned
                    ):
                        waits = list(si.on_wait)
                        for w in waits[:-1]:
                            ev = mybir.InstEventSemaphore(
                                name=f"I-{self.nc.next_id()}", ins=[], outs=[]
                            )
                            ev.engine = eng
                            ev.sync_info = mybir.SyncInfo(
                                on_wait=[w], on_update=[]
                            )
                            out.append(ev)
                        si.on_wait = waits[-1:]
                    out.append(inst)
                insts[:] = out
            return super()._lower_ordered_insts(ordered)

        # Same 1-wait limit applies to the tail drain; split its waits
        # into standalone wait instructions.
        def _drain_and_barrier(self, tick_clock, wait_clock):
            drain_inst = self.nc.sync.drain()
            wait_clock.add_sem_waits(
                drain_inst.ins, ScopedClock({None: tick_clock.global_clock})
            )
            si = drain_inst.ins.sync_info
            waits = list(si.on_wait) if si and si.on_wait else []
            if len(waits) > 1:
                si.on_wait = waits[:1]
                name2sem = {}
                for s in self.sems.allocated().values():
                    name2sem[getattr(s, "name", None) or str(s)] = s
                for w in waits[1:]:
                    self.nc.sync.wait_ge(name2sem[w.ant_name], w.wait_value)
            self.nc.all_engine_barrier()
            popped = self.nc._tile_sem_poison_stack.pop()
            assert popped is self._sem_poison
            self.nc.clear_and_free_semaphores(list(self.sems.allocated().values()))
            self.nc.all_engine_barrier()

    return TC


def build_program(causal: bool):
    import concourse.bass as bass
    import concourse.mybir as mybir
    import concourse.tile as tile

    dt = mybir.dt
    AF = mybir.ActivationFunctionType
    TC = _make_tc_class(tile)

    nc = bass.Bass("TRN2", target_bir_lowering=False, debug=False, num_devices=NC)

    xqT = nc.dram_tensor("xqT", [DM, TQ], dt.bfloat16, kind="ExternalInput")
    xkvT = nc.dram_tensor("xkvT", [DM, TKV], dt.bfloat16, kind="ExternalInput")
    wq_d = nc.dram_tensor("wq", [DM, HL * DQ], dt.bfloat16, kind="ExternalInput")
    wk_d = nc.dram_tensor("wk", [DM, HL * DQ], dt.bfloat16, kind="ExternalInput")
    wv_d = nc.dram_tensor("wv", [DM, HL * DV], dt.bfloat16, kind="ExternalInput")
    wo_d = nc.dram_tensor("wo", [HL * DV, DM], dt.bfloat16, kind="ExternalInput")
    bq_d = nc.dram_tensor("bqp", [P, NHP], dt.float32, kind="ExternalInput")
    bk_d = nc.dram_tensor("bkp", [P, NHP], dt.float32, kind="ExternalInput")
    pad_d = nc.dram_tensor("pad", [P, NKT], dt.float32, kind="ExternalInput")
    msk_d = nc.dram_tensor("msk", [P, 4 * SB], dt.bfloat16, kind="ExternalInput")
    one_d = nc.dram_tensor("one64", [P, HL * 64], dt.bfloat16, kind="ExternalInput")
    out_d = nc.dram_tensor("out", [TQ, DM], dt.float32, kind="ExternalOutput")

    with TC(nc) as tc:
        with (
            tc.tile_pool(name="res", bufs=1) as res,
            tc.tile_pool(name="xp", bufs=8) as xp,
            tc.tile_pool(name="ptp", bufs=4) as ptp,
            tc.tile_pool(name="atp", bufs=2) as atp,
            tc.tile_pool(name="rcp", bufs=2) as rcp,
            tc.tile_pool(name="ps_proj", bufs=2, space="PSUM") as ps_proj,
            tc.tile_pool(name="ps_s", bufs=3, space="PSUM") as ps_s,
            tc.tile_pool(name="ps_at", bufs=2, space="PSUM") as ps_at,
            tc.tile_pool(name="ps_bc", bufs=1, space="PSUM") as ps_bc,
        ):
            # ---- resident constants / weights ----
            wq_t, wk_t, wv_t = [], [], []
            for k in range(NDM):
                t = res.tile([P, HL * DQ], dt.bfloat16, tag=f"wq{k}", name=f"wq{k}")
                nc.sync.dma_start(t[:], wq_d.ap()[k * P:(k + 1) * P, :])
                wq_t.append(t)
                t = res.tile([P, HL * DQ], dt.bfloat16, tag=f"wk{k}", name=f"wk{k}")
                nc.sync.dma_start(t[:], wk_d.ap()[k * P:(k + 1) * P, :])
                wk_t.append(t)
                t = res.tile([P, HL * DV], dt.bfloat16, tag=f"wv{k}", name=f"wv{k}")
                nc.sync.dma_start(t[:], wv_d.ap()[k * P:(k + 1) * P, :])
                wv_t.append(t)
            wo_t = []
            for hp in range(NHP):
                t = res.tile([P, DM], dt.bfloat16, tag=f"wo{hp}", name=f"wo{hp}")
                nc.sync.dma_start(t[:], wo_d.ap()[hp * P:(hp + 1) * P, :])
                wo_t.append(t)
            bq_t = res.tile([P, NHP], dt.float32, tag="bq", name="bq_t")
            nc.sync.dma_start(bq_t[:], bq_d.ap()[:, :])
            bk_t = res.tile([P, NHP], dt.float32, tag="bk", name="bk_t")
            nc.sync.dma_start(bk_t[:], bk_d.ap()[:, :])
            pad_t = res.tile([P, NKT], dt.float32, tag="pad", name="pad_t")
            nc.sync.dma_start(pad_t[:], pad_d.ap()[:, :])
            msk_t = res.tile([P, 4 * SB], dt.bfloat16, tag="msk", name="msk_t")
            nc.sync.dma_start(msk_t[:], msk_d.ap()[:, :])
            sel_t = res.tile([P, HL * 64], dt.bfloat16, tag="sel", name="sel_t")
            nc.sync.dma_start(sel_t[:], one_d.ap()[:, :])


            # ---- x^T streams (xq slots reused by xkv) ----
            xq_t = []
            for k in range(NDM):
                t = xp.tile([P, TQ], dt.bfloat16, tag="xT", name="xT")
                nc.sync.dma_start(t[:], xqT.ap()[k * P:(k + 1) * P, :])
                xq_t.append(t)

            # ---- Q^T projection: qT[hp] = [128(2 heads x 64), TQ] bf16 ----
            qT = [res.tile([P, TQ], dt.bfloat16, tag=f"qT{hp}", name=f"qT{hp}") for hp in range(NHP)]
            for qsb in range(NQSB):
                for hp in range(NHP):
                    ps = ps_proj.tile([P, SB], dt.float32, tag="pp", name="pp")
                    for k in range(NDM):
                        nc.tensor.matmul(
                            ps[:],
                            wq_t[k][:, hp * P:(hp + 1) * P],
                            xq_t[k][:, qsb * SB:(qsb + 1) * SB],
                            start=(k == 0), stop=(k == NDM - 1),
                        )
                    with nc.allow_low_precision(reason="bf16 Q"):
                        nc.vector.tensor_scalar_add(
                            qT[hp][:, qsb * SB:(qsb + 1) * SB], ps[:],
                            bq_t[:, hp:hp + 1],
                        )

            xkv_t = []
            for k in range(NDM):
                t = xp.tile([P, TKV], dt.bfloat16, tag="xT", name="xT")
                nc.sync.dma_start(t[:], xkvT.ap()[k * P:(k + 1) * P, :])
                xkv_t.append(t)

            # ---- K^T and V projections (kv-chunk major so attention can
            # start early). V layout: per kv tile [128, 8 heads x 65]
            # (64 V cols + 1 ones col per head) for the PV+rowsum matmul. ----
            kT = [res.tile([P, TKV], dt.bfloat16, tag=f"kT{hp}", name=f"kT{hp}") for hp in range(NHP)]
            v_t = [res.tile([P, HL * 65], dt.bfloat16, tag=f"v{vt}", name=f"v{vt}") for vt in range(NKT)]
            for nj in range(NQSB):
                for hp in range(NHP):
                    ps = ps_proj.tile([P, SB], dt.float32, tag="pp", name="pp")
                    for k in range(NDM):
                        nc.tensor.matmul(
                            ps[:],
                            wk_t[k][:, hp * P:(hp + 1) * P],
                            xkv_t[k][:, nj * SB:(nj + 1) * SB],
                            start=(k == 0), stop=(k == NDM - 1),
                        )
                    with nc.allow_low_precision(reason="bf16 K"):
                        nc.vector.tensor_scalar_add(
                            kT[hp][:, nj * SB:(nj + 1) * SB], ps[:],
                            bk_t[:, hp:hp + 1],
                        )
                for vt in range(nj * 4, nj * 4 + 4):
                    ps = ps_proj.tile([P, SB], dt.float32, tag="pp", name="pp")
                    for k in range(NDM):
                        nc.tensor.matmul(
                            ps[:],
                            xkv_t[k][:, vt * P:(vt + 1) * P],
                            wv_t[k][:],
                            start=(k == 0), stop=(k == NDM - 1),
                        )
                    vtile = v_t[vt]
                    nc.vector.tensor_copy(
                        vtile[:].rearrange("p (h d) -> p h d", d=65)[:, :, 0:64],
                        ps[:].rearrange("p (h d) -> p h d", d=64),
                    )
                    nc.gpsimd.memset(
                        vtile[:].rearrange("p (h d) -> p h d", d=65)[:, :, 64:65], 1.0
                    )

            # ---- attention + output projection, per q super-block ----
            def emit_finalize(qsb, at_tiles, an_tiles, rc):
                for h in range(HL):
                    hp, off = h // 2, (h % 2) * 64
                    # broadcast head h's recip row across 64 partitions via
                    # PE: sel[:, h*64:(h+1)*64] is 1.0 on row 32*(h%4)
                    bc = ps_bc.tile([64, SB], dt.float32, tag="bc", name="bc")
                    cg = (h // 4) * SB
                    nc.tensor.matmul(
                        bc[:], sel_t[:, h * 64:(h + 1) * 64],
                        rc[:, cg:cg + SB],
                        start=True, stop=True,
                    )
                    nc.vector.tensor_mul(
                        at_tiles[hp][off:off + 64, :], an_tiles[h][:], bc[:]
                    )

            def emit_outproj(qsb, at_tiles):
                for qt in range(4):
                    for col in range(2):
                        ps = ps_proj.tile([P, SB], dt.float32, tag="pp", name="pp")
                        for hp in range(NHP):
                            nc.tensor.matmul(
                                ps[:],
                                at_tiles[hp][:, qt * P:(qt + 1) * P],
                                wo_t[hp][:, col * SB:(col + 1) * SB],
                                start=(hp == 0), stop=(hp == NHP - 1),
                            )
                        ost = rcp.tile([P, SB], dt.float32, tag="ost", name="ost", bufs=3)
                        nc.vector.tensor_copy(ost[:], ps[:])
                        r0 = qsb * SB + qt * P
                        nc.sync.dma_start(
                            out_d.ap()[r0:r0 + P, col * SB:(col + 1) * SB], ost[:]
                        )

            prev_blk = None
            for qsb in range(NQSB):
                at_tiles = [
                    atp.tile([P, SB], dt.bfloat16, tag=f"attnT{hp}", name=f"attnT{hp}")
                    for hp in range(NHP)
                ]
                kt_max = 4 * qsb + 4 if causal else NKT
                # normalizer rows gathered at partitions {0,32,64,96} x 2
                # column groups (DVE partition offsets must be 32-aligned);
                # filler 1.0 keeps the reciprocal finite on unused rows
                sums = rcp.tile([P, 2 * SB], dt.float32, tag="sums", name="sums")
                nc.gpsimd.memset(sums[:], 1.0)
                an_tiles = []
                for hp in range(NHP):
                    # the pair's S matmuls target disjoint PE row groups
                    # (0-63 / 64-127) and run concurrently when adjacent
                    aps2 = [
                        ps_at.tile([65, SB], dt.float32, tag="at", name="at")
                        for _ in range(2)
                    ]
                    for kt in range(kt_max):
                        # diagonal blocks (j>=1): only q columns >= 128*j can
                        # be unmasked -> trim the left columns entirely
                        j = kt - 4 * qsb if causal else -1
                        c0 = 128 * j if j > 0 else 0
                        pts = []
                        for e in range(2):
                            off = e * 64
                            sps = ps_s.tile([P, SB], dt.float32, tag="s",
                                            name="s")
                            nc.tensor.matmul(
                                sps[:, c0:SB],
                                kT[hp][off:off + 64, kt * P:(kt + 1) * P],
                                qT[hp][off:off + 64,
                                       qsb * SB + c0:(qsb + 1) * SB],
                                start=True, stop=True,
                            )
                            pt = ptp.tile([P, SB], dt.bfloat16, tag="pT",
                                          name="pT")
                            nc.scalar.activation(
                                pt[:, c0:SB], sps[:, c0:SB], AF.Exp,
                                scale=0.125, bias=pad_t[:, kt:kt + 1],
                            )
                            if causal and j >= 0:
                                nc.vector.tensor_mul(
                                    pt[:, c0:SB], pt[:, c0:SB],
                                    msk_t[:, j * SB + c0:(j + 1) * SB],
                                )
                            pts.append(pt)
                        for e in range(2):
                            h = 2 * hp + e
                            nc.tensor.matmul(
                                aps2[e][:, c0:SB],
                                v_t[kt][:, h * 65:h * 65 + 65],
                                pts[e][:, c0:SB],
                                start=(kt == 0), stop=(kt == kt_max - 1),
                            )
                    for e in range(2):
                        h = 2 * hp + e
                        # stage numerator + normalizer row to SBUF, free psum
                        an = rcp.tile([64, SB], dt.bfloat16, tag=f"an{h}",
                                      name=f"an{h}")
                        nc.vector.tensor_copy(an[:], aps2[e][0:64, :])
                        r, cg = 32 * (h % 4), (h // 4) * SB
                        nc.vector.tensor_copy(
                            sums[r:r + 1, cg:cg + SB], aps2[e][64:65, :]
                        )
                        an_tiles.append(an)
                # one batched reciprocal for all 8 heads of this q-block
                rc = rcp.tile([P, 2 * SB], dt.bfloat16, tag="rc", name="rc")
                with nc.allow_low_precision(reason="bf16 softmax recip"):
                    nc.vector.reciprocal(rc[:], sums[:])
                # divisions + output projection for the PREVIOUS q block are
                # emitted here, after this block's attention: its reciprocal
                # has long finished, so the PE stream never stalls on it
                if prev_blk is not None:
                    emit_finalize(*prev_blk)
                    emit_outproj(prev_blk[0], prev_blk[1])
                prev_blk = (qsb, at_tiles, an_tiles, rc)
            emit_finalize(*prev_blk)
            emit_outproj(prev_blk[0], prev_blk[1])
    return nc


def _get_program(causal: bool):
    key = bool(causal)
    if key not in _programs:
        _programs[key] = build_program(key)
    return _programs[key]


def kernel(**inputs):
    from concourse.bass_utils import run_bass_kernel_spmd

    xq = np.asarray(inputs["query_sequence"], dtype=np.float32)
    xkv = np.asarray(inputs["key_value_sequence"], dtype=np.float32)
    pmask = np.asarray(inputs["key_value_padding_mask"])
    Wq = np.asarray(inputs["Wq"], dtype=np.float32)
    bq = np.asarray(inputs["bq"], dtype=np.float32)
    Wkv = np.asarray(inputs["Wkv"], dtype=np.float32)
    bkv = np.asarray(inputs["bkv"], dtype=np.float32)
    Wo = np.asarray(inputs["Wo"], dtype=np.float32)
    bo = np.asarray(inputs["bo"], dtype=np.float32)
    causal = bool(np.asarray(inputs["apply_causal_mask"]))

    nc = _get_program(causal)

    Wk_full = Wkv[:, : H * DQ]
    Wv_full = Wkv[:, H * DQ:]
    bk_full = bkv[: H * DQ]
    bv_full = bkv[H * DQ:]

    # causal diagonal masks: mask_j[kv, q] = 1 if q >= kv + 128*j (within a
    # [128 kv, 512 q] block at kv-tile offset j of the q super-block)
    kvi = np.arange(P)[:, None]
    qi = np.arange(SB)[None, :]
    msk = np.concatenate(
        [(qi >= kvi + P * j).astype(np.float32) for j in range(4)], axis=1
    ).astype(bf16)
    # head-row selector: sel[:, h*64:(h+1)*64] = 1.0 on row 32*(h%4) else 0
    sel = np.zeros((P, HL * 64), np.float32)
    for h in range(HL):
        sel[32 * (h % 4), h * 64:(h + 1) * 64] = 1.0
    sel = sel.astype(bf16)

    in_maps = []
    for c in range(NC):
        b, g = divmod(c, 2)
        hs = slice(g * HL * DQ, (g + 1) * HL * DQ)
        pb = np.where(pmask[b], np.float32(-1e30), np.float32(0.0))
        in_maps.append({
            "xqT": np.ascontiguousarray(xq[b].T).astype(bf16),
            "xkvT": np.ascontiguousarray(xkv[b].T).astype(bf16),
            "wq": np.ascontiguousarray(Wq[:, hs]).astype(bf16),
            "wk": np.ascontiguousarray(Wk_full[:, hs]).astype(bf16),
            "wv": np.ascontiguousarray(Wv_full[:, hs]).astype(bf16),
            "wo": np.ascontiguousarray(Wo[hs, :]).astype(bf16),
            "bqp": np.ascontiguousarray(bq[hs].reshape(NHP, P).T),
            "bkp": np.ascontiguousarray(bk_full[hs].reshape(NHP, P).T),
            "pad": np.ascontiguousarray(pb.reshape(NKT, P).T),
            "msk": msk,
            "one64": sel,
        })

    global _last_in_maps
    _last_in_maps = in_maps
    res = run_bass_kernel_spmd(nc, in_maps, core_ids=list(range(NC)))

    host_bias = bo + bv_full @ Wo  # softmax rows sum to 1 -> V-bias is additive
    out = np.empty((B, TQ, DM), np.float32)
    for b in range(B):
        out[b] = res.results[2 * b]["out"] + res.results[2 * b + 1]["out"] + host_bias
    return out
